# revision 8
# baseline (speedup 1.0000x reference)
"""AttentionReadout kernel for Trainium2 (8 NeuronCores, data-parallel by chunk).

Reference computation (per full input):
    scores = (tanh(x @ W1 + b1) @ W2)[:, 0]          # [N]
    chunk_id = batch // 32                            # 32 graphs per chunk
    w = softmax of scores within each chunk           # [N]
    out = segment_sum(w[:, None] * x, batch)          # [4096, 256]

Shapes: x [262144, 256] f32, batch [262144] i64 (sorted, uniform: 64
nodes/graph), W1 [256,256], b1 [256], W2 [256,1].

Strategy (per core, 32768 nodes = 16 chunks of 2048 nodes):
  - host: cast x to bf16; ship both natural layout (pooling rhs) and
    transposed layout (MLP rhs); replicate tiny weights.
  - device, per chunk:
      hT = W1.T @ xT          (PE, bf16, psum f32)
      th = tanh(hT + b1)      (ACT, psum->sbuf bf16)
      s[n] = th.T @ W2        (PE, tanh tile as stationary operand -> s in
                               node-partition layout [128,16] psum)
      e = exp(s), rowsum      (ACT fused accum_out)
      D = allreduce(rowsum)   (GPSIMD partition_all_reduce)
      w = e * (1/D)           (DVE)
      E[n, g] = w * mask      (DVE, mask precomputed on host)
      out[g,:] = E.T @ x      (PE, accumulate 16 node-tiles per chunk)
  - softmax max-subtraction is skipped: scores = tanh(.)@W2 are bounded by
    sum|W2| <= 16, so exp() cannot overflow in f32 and w = e/sum(e) is
    mathematically identical to the max-shifted form.
"""

import numpy as np
import ml_dtypes

import concourse.bass as bass
import concourse.bacc as bacc
import concourse.tile as tile
import concourse.mybir as mybir
import concourse.bass_isa as bass_isa
from concourse.bass_utils import run_bass_kernel_spmd

BF16 = mybir.dt.bfloat16
F32 = mybir.dt.float32
NP_BF16 = ml_dtypes.bfloat16

N_CORES = 8
HIDDEN = 256
CHUNK_GRAPHS = 32
GRAPH_NODES = 64          # uniform: nodes per graph
TILE_NODES = 128          # nodes per node-tile (SBUF partition dim)
CHUNK_NODES = CHUNK_GRAPHS * GRAPH_NODES      # 2048
TILES_PER_CHUNK = CHUNK_NODES // TILE_NODES   # 16
BLOCKS_PER_CHUNK = 4                          # sub-blocks of 512 nodes
BLOCK_NODES = 512

_NC_CACHE = {}


def build_nc(n_chunks, repeat=1, out_name="out", salt=0):
    """Build the per-core Bass program (identical across cores)."""
    nc = bacc.Bacc("TRN2", target_bir_lowering=False, debug=False,
                   enable_asserts=False)

    nodes = n_chunks * CHUNK_NODES
    # DRAM I/O (per-core shard)
    x_nat_d = nc.dram_tensor(
        "x_nat", [n_chunks, TILE_NODES, TILES_PER_CHUNK, HIDDEN], BF16,
        kind="ExternalInput").ap()
    x_tr_d = nc.dram_tensor(
        "x_tr", [2, 128, nodes], BF16, kind="ExternalInput").ap()
    w1_d = nc.dram_tensor("w1", [128, 2, 2, 128], BF16,
                          kind="ExternalInput").ap()
    w2_d = nc.dram_tensor("w2", [128, 2], BF16, kind="ExternalInput").ap()
    b1_d = nc.dram_tensor("b1", [128, 2], F32, kind="ExternalInput").ap()
    mask_d = nc.dram_tensor(
        "maskw", [TILE_NODES, CHUNK_GRAPHS, TILES_PER_CHUNK], BF16,
        kind="ExternalInput").ap()
    out_d = nc.dram_tensor(
        out_name, [n_chunks * CHUNK_GRAPHS, HIDDEN], F32,
        kind="ExternalOutput").ap()
    if salt:
        # dummy input whose shape varies per build variant: defeats
        # executable-cache dedup between otherwise identical HLO programs
        nc.dram_tensor("salt", [1, salt], F32, kind="ExternalInput")

    with tile.TileContext(nc) as tc:
        with (
            tc.tile_pool(name="consts", bufs=1) as consts,
            tc.tile_pool(name="xpool", bufs=4) as xpool,
            tc.tile_pool(name="xtpool", bufs=3) as xtpool,
            tc.tile_pool(name="thpool", bufs=8) as thpool,
            tc.tile_pool(name="epool", bufs=3) as epool,
            tc.tile_pool(name="opool", bufs=2) as opool,
            tc.tile_pool(name="hpsum", bufs=2, space="PSUM") as hpsum,
            tc.tile_pool(name="spsum", bufs=2, space="PSUM") as spsum,
            tc.tile_pool(name="ppsum", bufs=2, space="PSUM") as ppsum,
        ):
            w1_sb = consts.tile([128, 2, 2, 128], BF16)
            nc.sync.dma_start(out=w1_sb, in_=w1_d)
            w2_sb = consts.tile([128, 2], BF16)
            nc.sync.dma_start(out=w2_sb, in_=w2_d)
            b1_sb = consts.tile([128, 2], F32)
            nc.sync.dma_start(out=b1_sb, in_=b1_d)
            mask_sb = consts.tile([TILE_NODES, CHUNK_GRAPHS, TILES_PER_CHUNK],
                                  BF16)
            nc.sync.dma_start(out=mask_sb, in_=mask_d)

            # Software pipeline, 3 chunks deep on the PE:
            #   iteration c emits, round-robin per node-tile:
            #     MLP matmuls of chunk c   (F=512 streams)
            #     score matmuls of c-1     (weight-load bound; hides under MLP)
            #     pooling matmuls of c-2   (F=256 streams)
            #   plus tanh(c) on ACT and the softmax chain (c-1) on
            #   ACT/GPSIMD/DVE, which runs while chunk c streams.
            st = {}  # per-chunk live tiles  (reassigned per repeat)

            def emit_loads(c):
                x_sb = xpool.tile([TILE_NODES, TILES_PER_CHUNK, HIDDEN], BF16,
                                  tag="x")
                nc.sync.dma_start(out=x_sb, in_=x_nat_d[c])
                xt_sb = xtpool.tile([128, 2, CHUNK_NODES], BF16, tag="xt")
                nc.sync.dma_start(
                    out=xt_sb,
                    in_=x_tr_d[:, :, c * CHUNK_NODES:(c + 1) * CHUNK_NODES]
                    .transpose([1, 0, 2]))
                st[c] = {"x": x_sb, "xt": xt_sb, "th": {}}

            def mlp_gen(c):
                """Yields after each MLP matmul (16 total)."""
                xt_sb = st[c]["xt"]
                s_ps = spsum.tile([128, TILES_PER_CHUNK], F32, tag="s",
                                  name=f"s_ps{c}")
                st[c]["s"] = s_ps
                for bp in range(2):          # block pair: nodes [bp*1024, ...)
                    for mt in range(2):
                        h_ps = hpsum.tile([128, 2, BLOCK_NODES], F32, tag="h",
                                          name=f"h_ps{c}_{bp}_{mt}")
                        for bb in range(2):
                            nlo = (2 * bp + bb) * BLOCK_NODES
                            for kt in range(2):
                                nc.tensor.matmul(
                                    h_ps[:, bb, :], w1_sb[:, kt, mt, :],
                                    xt_sb[:, kt, nlo:nlo + BLOCK_NODES],
                                    start=(kt == 0), stop=(kt == 1))
                                yield
                        th = thpool.tile([128, 2, BLOCK_NODES], BF16, tag="th",
                                         name=f"th{c}_{bp}_{mt}")
                        nc.scalar.activation(
                            th, h_ps, mybir.ActivationFunctionType.Tanh,
                            bias=b1_sb[:, mt:mt + 1], scale=1.0)
                        st[c]["th"][(bp, mt)] = th

            def score_ops(c):
                """16 closures; closure t emits the 2 accumulating matmuls
                producing s[:, t] (tanh tile as stationary operand)."""
                ops = []
                for t in range(TILES_PER_CHUNK):
                    b, tl = divmod(t, 4)
                    bp, bb = divmod(b, 2)

                    def op(t=t, bp=bp, bb=bb, tl=tl, c=c):
                        s_ps = st[c]["s"]
                        for mt in range(2):
                            th = st[c]["th"][(bp, mt)]
                            nc.tensor.matmul(
                                s_ps[:, t:t + 1],
                                th[:, bb, tl * 128:(tl + 1) * 128],
                                w2_sb[:, mt:mt + 1],
                                start=(mt == 0), stop=(mt == 1))
                    ops.append(op)
                return ops

            def emit_softmax(c):
                s_ps = st[c]["s"]
                e_sb = epool.tile([128, TILES_PER_CHUNK], BF16, tag="e")
                acc = epool.tile([128, 1], F32, tag="acc")
                nc.scalar.activation(
                    e_sb, s_ps, mybir.ActivationFunctionType.Exp,
                    accum_out=acc)
                dsum = epool.tile([128, 1], F32, tag="dsum")
                nc.gpsimd.partition_all_reduce(
                    dsum, acc, 128, bass_isa.ReduceOp.add)
                rden = epool.tile([128, 1], F32, tag="rden")
                nc.vector.reciprocal(rden, dsum)
                w_sb = epool.tile([128, TILES_PER_CHUNK], BF16, tag="w")
                nc.vector.tensor_scalar_mul(w_sb, e_sb, rden)
                e_full = epool.tile(
                    [TILE_NODES, CHUNK_GRAPHS, TILES_PER_CHUNK], BF16,
                    tag="efull")
                w_bc = w_sb.unsqueeze(1).broadcast_to(
                    [TILE_NODES, CHUNK_GRAPHS, TILES_PER_CHUNK])
                nc.vector.tensor_mul(e_full, w_bc, mask_sb)
                st[c]["E"] = e_full
                p_ps = ppsum.tile([CHUNK_GRAPHS, HIDDEN], F32, tag="p",
                                  name=f"p_ps{c}")
                st[c]["p"] = p_ps

            def pool_ops(c):
                ops = []
                for t in range(TILES_PER_CHUNK):
                    def op(t=t, c=c):
                        nc.tensor.matmul(
                            st[c]["p"], st[c]["E"][:, :, t], st[c]["x"][:, t, :],
                            start=(t == 0), stop=(t == TILES_PER_CHUNK - 1))
                    ops.append(op)
                return ops

            def emit_store(c):
                o_sb = opool.tile([CHUNK_GRAPHS, HIDDEN], F32, tag="o")
                nc.vector.tensor_copy(o_sb, st[c]["p"])
                nc.sync.dma_start(
                    out=out_d[c * CHUNK_GRAPHS:(c + 1) * CHUNK_GRAPHS, :],
                    in_=o_sb)
                # release references that are no longer needed
                del st[c]

            for _rep in range(repeat):
                emit_loads(0)
                for c in range(n_chunks + 2):
                    if c + 1 < n_chunks:
                        emit_loads(c + 1)
                    mg = mlp_gen(c) if c < n_chunks else None
                    sops = score_ops(c - 1) if 1 <= c <= n_chunks else None
                    pops = pool_ops(c - 2) if c >= 2 else None
                    for i in range(TILES_PER_CHUNK):
                        if mg is not None:
                            next(mg, None)
                        if sops is not None:
                            sops[i]()
                        if pops is not None:
                            pops[i]()
                    if mg is not None:
                        for _ in mg:  # drain remaining (none expected)
                            pass
                    if 1 <= c <= n_chunks:
                        emit_softmax(c - 1)
                    if c >= 2:
                        emit_store(c - 2)

    nc.compile()
    return nc


def _prep_inputs(x, W1, b1, W2, n_chunks_per_core):
    """Host-side marshalling: bf16 cast, layouts, masks. Returns in_maps."""
    N, H = x.shape
    nodes_per_core = n_chunks_per_core * CHUNK_NODES

    xb = np.asarray(x).astype(NP_BF16)

    # natural layout: [core, chunk, p, t, h]
    x_nat = np.ascontiguousarray(
        xb.reshape(N_CORES, n_chunks_per_core, TILES_PER_CHUNK, TILE_NODES, H)
        .transpose(0, 1, 3, 2, 4))
    # transposed layout: [core, kt, q, n_local]
    x_tr = np.ascontiguousarray(
        xb.reshape(N_CORES, nodes_per_core, H).transpose(0, 2, 1)
        .reshape(N_CORES, 2, 128, nodes_per_core))

    W1b = np.asarray(W1).astype(NP_BF16)     # [hin, hout]
    w1_host = np.ascontiguousarray(
        W1b.reshape(2, 128, 2, 128).transpose(1, 0, 2, 3))  # [p, kt, mt, j]
    w2_host = np.ascontiguousarray(
        np.asarray(W2).astype(NP_BF16).reshape(2, 128).T)   # [p, mt]
    b1_host = np.ascontiguousarray(
        np.asarray(b1).astype(np.float32).reshape(2, 128).T)  # [p, mt]

    # mask[p, g, t] = 1 iff node (t, p) of a chunk belongs to graph g
    p_idx = np.arange(TILE_NODES)
    t_idx = np.arange(TILES_PER_CHUNK)
    g_of_pt = 2 * t_idx[None, :] + p_idx[:, None] // GRAPH_NODES  # [p, t]
    mask_host = (g_of_pt[:, None, :] ==
                 np.arange(CHUNK_GRAPHS)[None, :, None]).astype(NP_BF16)

    in_maps = []
    for core in range(N_CORES):
        in_maps.append({
            "x_nat": x_nat[core],
            "x_tr": x_tr[core],
            "w1": w1_host,
            "w2": w2_host,
            "b1": b1_host,
            "maskw": mask_host,
        })
    return in_maps


def _reference_numpy(x, batch, W1, b1, W2):
    """Fallback for non-uniform batch layouts: straight numpy."""
    x = np.asarray(x, dtype=np.float64)
    batch = np.asarray(batch).astype(np.int64)
    n_graphs = int(batch.max()) + 1
    scores = np.tanh(x @ np.asarray(W1, np.float64) +
                     np.asarray(b1, np.float64)) @ np.asarray(W2, np.float64)
    scores = scores[:, 0]
    chunk_id = batch // CHUNK_GRAPHS
    n_chunks = int(chunk_id.max()) + 1
    m = np.full(n_chunks, -np.inf)
    np.maximum.at(m, chunk_id, scores)
    e = np.exp(scores - m[chunk_id])
    denom = np.zeros(n_chunks)
    np.add.at(denom, chunk_id, e)
    w = e / denom[chunk_id]
    out = np.zeros((n_graphs, x.shape[1]))
    np.add.at(out, batch, w[:, None] * x)
    return out.astype(np.float32)


def kernel(x, batch, W1, b1, W2, trace=False):
    x = np.asarray(x)
    batch = np.asarray(batch)
    N, H = x.shape
    n_graphs = int(batch[-1]) + 1

    # This kernel is specialized for the uniform sorted batch that the
    # reference generator produces (64 nodes per graph). Anything else
    # falls back to a host computation.
    expected = (np.arange(N, dtype=np.int64) * n_graphs) // N
    if (H != HIDDEN or N % (N_CORES * CHUNK_NODES) != 0
            or n_graphs % (N_CORES * CHUNK_GRAPHS) != 0
            or not np.array_equal(batch.astype(np.int64), expected)):
        return _reference_numpy(x, batch, W1, b1, W2)

    n_chunks_per_core = N // (N_CORES * CHUNK_NODES)

    key = n_chunks_per_core
    if key not in _NC_CACHE:
        _NC_CACHE[key] = build_nc(n_chunks_per_core)
    nc = _NC_CACHE[key]

    in_maps = _prep_inputs(x, W1, b1, W2, n_chunks_per_core)
    try:
        res = run_bass_kernel_spmd(nc, in_maps, core_ids=list(range(N_CORES)),
                                   trace=trace)
    except ModuleNotFoundError:
        # NTFF trace hooks unavailable in this environment
        res = run_bass_kernel_spmd(nc, in_maps, core_ids=list(range(N_CORES)),
                                   trace=False)
    out = np.concatenate([r["out"] for r in res.results], axis=0)
    if trace:
        kernel.last_results = res
    return out.astype(np.float32)


# revision 9
# speedup vs baseline: 58465.0441x; 58465.0441x over previous
"""AttentionReadout kernel for Trainium2 (8 NeuronCores, data-parallel by chunk).

Reference computation (per full input):
    scores = (tanh(x @ W1 + b1) @ W2)[:, 0]          # [N]
    chunk_id = batch // 32                            # 32 graphs per chunk
    w = softmax of scores within each chunk           # [N]
    out = segment_sum(w[:, None] * x, batch)          # [4096, 256]

Shapes: x [262144, 256] f32, batch [262144] i64 (sorted, uniform: 64
nodes/graph), W1 [256,256], b1 [256], W2 [256,1].

Strategy (per core, 32768 nodes = 16 chunks of 2048 nodes):
  - host: cast x to bf16; ship both natural layout (pooling rhs) and
    transposed layout (MLP rhs); replicate tiny weights.
  - device, per chunk:
      hT = W1.T @ xT          (PE, bf16, psum f32)
      th = tanh(hT + b1)      (ACT, psum->sbuf bf16)
      s[n] = th.T @ W2        (PE, tanh tile as stationary operand -> s in
                               node-partition layout [128,16] psum)
      e = exp(s), rowsum      (ACT fused accum_out)
      D = allreduce(rowsum)   (GPSIMD partition_all_reduce)
      w = e * (1/D)           (DVE)
      E[n, g] = w * mask      (DVE, mask precomputed on host)
      out[g,:] = E.T @ x      (PE, accumulate 16 node-tiles per chunk)
  - softmax max-subtraction is skipped: scores = tanh(.)@W2 are bounded by
    sum|W2| <= 16, so exp() cannot overflow in f32 and w = e/sum(e) is
    mathematically identical to the max-shifted form.
"""

import numpy as np
import ml_dtypes

import concourse.bass as bass
import concourse.bacc as bacc
import concourse.tile as tile
import concourse.mybir as mybir
import concourse.bass_isa as bass_isa
from concourse.bass_utils import run_bass_kernel_spmd

BF16 = mybir.dt.bfloat16
F32 = mybir.dt.float32
NP_BF16 = ml_dtypes.bfloat16

N_CORES = 8
HIDDEN = 256
CHUNK_GRAPHS = 32
GRAPH_NODES = 64          # uniform: nodes per graph
TILE_NODES = 128          # nodes per node-tile (SBUF partition dim)
CHUNK_NODES = CHUNK_GRAPHS * GRAPH_NODES      # 2048
TILES_PER_CHUNK = CHUNK_NODES // TILE_NODES   # 16
BLOCKS_PER_CHUNK = 4                          # sub-blocks of 512 nodes
BLOCK_NODES = 512

_NC_CACHE = {}


def build_nc(n_chunks, repeat=1, out_name="out", salt=0):
    """Build the per-core Bass program (identical across cores)."""
    nc = bacc.Bacc("TRN2", target_bir_lowering=False, debug=False,
                   enable_asserts=False)

    nodes = n_chunks * CHUNK_NODES
    # DRAM I/O (per-core shard)
    x_nat_d = nc.dram_tensor(
        "x_nat", [n_chunks, TILE_NODES, TILES_PER_CHUNK, HIDDEN], BF16,
        kind="ExternalInput").ap()
    x_tr_d = nc.dram_tensor(
        "x_tr", [2, 128, nodes], BF16, kind="ExternalInput").ap()
    w1_d = nc.dram_tensor("w1", [128, 2, 2, 128], BF16,
                          kind="ExternalInput").ap()
    w2_d = nc.dram_tensor("w2", [128, 2], BF16, kind="ExternalInput").ap()
    b1_d = nc.dram_tensor("b1", [128, 2], F32, kind="ExternalInput").ap()
    mask_d = nc.dram_tensor(
        "maskw", [TILE_NODES, CHUNK_GRAPHS, TILES_PER_CHUNK], BF16,
        kind="ExternalInput").ap()
    out_d = nc.dram_tensor(
        out_name, [n_chunks * CHUNK_GRAPHS, HIDDEN], F32,
        kind="ExternalOutput").ap()
    if salt:
        # dummy input whose shape varies per build variant: defeats
        # executable-cache dedup between otherwise identical HLO programs
        nc.dram_tensor("salt", [1, salt], F32, kind="ExternalInput")

    with tile.TileContext(nc) as tc:
        with (
            tc.tile_pool(name="consts", bufs=1) as consts,
            tc.tile_pool(name="xpool", bufs=4) as xpool,
            tc.tile_pool(name="xtpool", bufs=3) as xtpool,
            tc.tile_pool(name="thpool", bufs=8) as thpool,
            tc.tile_pool(name="epool", bufs=3) as epool,
            tc.tile_pool(name="opool", bufs=2) as opool,
            tc.tile_pool(name="hpsum", bufs=2, space="PSUM") as hpsum,
            tc.tile_pool(name="spsum", bufs=2, space="PSUM") as spsum,
            tc.tile_pool(name="ppsum", bufs=2, space="PSUM") as ppsum,
        ):
            w1_sb = consts.tile([128, 2, 2, 128], BF16)
            nc.sync.dma_start(out=w1_sb, in_=w1_d)
            w2_sb = consts.tile([128, 2], BF16)
            nc.sync.dma_start(out=w2_sb, in_=w2_d)
            b1_sb = consts.tile([128, 2], F32)
            nc.sync.dma_start(out=b1_sb, in_=b1_d)
            mask_sb = consts.tile([TILE_NODES, CHUNK_GRAPHS, TILES_PER_CHUNK],
                                  BF16)
            nc.sync.dma_start(out=mask_sb, in_=mask_d)

            # Software pipeline, 3 chunks deep on the PE:
            #   iteration c emits, round-robin per node-tile:
            #     MLP matmuls of chunk c   (F=512 streams)
            #     score matmuls of c-1     (weight-load bound; hides under MLP)
            #     pooling matmuls of c-2   (F=256 streams)
            #   plus tanh(c) on ACT and the softmax chain (c-1) on
            #   ACT/GPSIMD/DVE, which runs while chunk c streams.
            st = {}  # per-chunk live tiles  (reassigned per repeat)

            def emit_loads(c):
                x_sb = xpool.tile([TILE_NODES, TILES_PER_CHUNK, HIDDEN], BF16,
                                  tag="x")
                nc.sync.dma_start(out=x_sb, in_=x_nat_d[c])
                xt_sb = xtpool.tile([128, 2, CHUNK_NODES], BF16, tag="xt")
                nc.sync.dma_start(
                    out=xt_sb,
                    in_=x_tr_d[:, :, c * CHUNK_NODES:(c + 1) * CHUNK_NODES]
                    .transpose([1, 0, 2]))
                st[c] = {"x": x_sb, "xt": xt_sb, "th": {}}

            def mlp_gen(c):
                """Yields after each MLP matmul (16 total)."""
                xt_sb = st[c]["xt"]
                s_ps = spsum.tile([128, TILES_PER_CHUNK], F32, tag="s",
                                  name=f"s_ps{c}")
                st[c]["s"] = s_ps
                for bp in range(2):          # block pair: nodes [bp*1024, ...)
                    for mt in range(2):
                        h_ps = hpsum.tile([128, 2, BLOCK_NODES], F32, tag="h",
                                          name=f"h_ps{c}_{bp}_{mt}")
                        for bb in range(2):
                            nlo = (2 * bp + bb) * BLOCK_NODES
                            for kt in range(2):
                                nc.tensor.matmul(
                                    h_ps[:, bb, :], w1_sb[:, kt, mt, :],
                                    xt_sb[:, kt, nlo:nlo + BLOCK_NODES],
                                    start=(kt == 0), stop=(kt == 1))
                                yield
                        th = thpool.tile([128, 2, BLOCK_NODES], BF16, tag="th",
                                         name=f"th{c}_{bp}_{mt}")
                        nc.scalar.activation(
                            th, h_ps, mybir.ActivationFunctionType.Tanh,
                            bias=b1_sb[:, mt:mt + 1], scale=1.0)
                        st[c]["th"][(bp, mt)] = th

            def score_ops(c):
                """16 closures; closure t emits the 2 accumulating matmuls
                producing s[:, t] (tanh tile as stationary operand)."""
                ops = []
                for t in range(TILES_PER_CHUNK):
                    b, tl = divmod(t, 4)
                    bp, bb = divmod(b, 2)

                    def op(t=t, bp=bp, bb=bb, tl=tl, c=c):
                        s_ps = st[c]["s"]
                        for mt in range(2):
                            th = st[c]["th"][(bp, mt)]
                            nc.tensor.matmul(
                                s_ps[:, t:t + 1],
                                th[:, bb, tl * 128:(tl + 1) * 128],
                                w2_sb[:, mt:mt + 1],
                                start=(mt == 0), stop=(mt == 1))
                    ops.append(op)
                return ops

            def emit_softmax(c):
                s_ps = st[c]["s"]
                e_sb = epool.tile([128, TILES_PER_CHUNK], BF16, tag="e")
                acc = epool.tile([128, 1], F32, tag="acc")
                nc.scalar.activation(
                    e_sb, s_ps, mybir.ActivationFunctionType.Exp,
                    accum_out=acc)
                dsum = epool.tile([128, 1], F32, tag="dsum")
                nc.gpsimd.partition_all_reduce(
                    dsum, acc, 128, bass_isa.ReduceOp.add)
                rden = epool.tile([128, 1], F32, tag="rden")
                nc.vector.reciprocal(rden, dsum)
                w_sb = epool.tile([128, TILES_PER_CHUNK], BF16, tag="w")
                nc.vector.tensor_scalar_mul(w_sb, e_sb, rden)
                e_full = epool.tile(
                    [TILE_NODES, CHUNK_GRAPHS, TILES_PER_CHUNK], BF16,
                    tag="efull")
                w_bc = w_sb.unsqueeze(1).broadcast_to(
                    [TILE_NODES, CHUNK_GRAPHS, TILES_PER_CHUNK])
                nc.vector.tensor_mul(e_full, w_bc, mask_sb)
                st[c]["E"] = e_full
                p_ps = ppsum.tile([CHUNK_GRAPHS, HIDDEN], F32, tag="p",
                                  name=f"p_ps{c}")
                st[c]["p"] = p_ps

            def pool_ops(c):
                ops = []
                for t in range(TILES_PER_CHUNK):
                    def op(t=t, c=c):
                        nc.tensor.matmul(
                            st[c]["p"], st[c]["E"][:, :, t], st[c]["x"][:, t, :],
                            start=(t == 0), stop=(t == TILES_PER_CHUNK - 1))
                    ops.append(op)
                return ops

            def emit_store(c):
                o_sb = opool.tile([CHUNK_GRAPHS, HIDDEN], F32, tag="o")
                nc.vector.tensor_copy(o_sb, st[c]["p"])
                nc.sync.dma_start(
                    out=out_d[c * CHUNK_GRAPHS:(c + 1) * CHUNK_GRAPHS, :],
                    in_=o_sb)
                # release references that are no longer needed
                del st[c]

            for _rep in range(repeat):
                emit_loads(0)
                for c in range(n_chunks + 2):
                    if c + 1 < n_chunks:
                        emit_loads(c + 1)
                    mg = mlp_gen(c) if c < n_chunks else None
                    sops = score_ops(c - 1) if 1 <= c <= n_chunks else None
                    pops = pool_ops(c - 2) if c >= 2 else None
                    for i in range(TILES_PER_CHUNK):
                        if mg is not None:
                            next(mg, None)
                        if sops is not None:
                            sops[i]()
                        if pops is not None:
                            pops[i]()
                    if mg is not None:
                        for _ in mg:  # drain remaining (none expected)
                            pass
                    if 1 <= c <= n_chunks:
                        emit_softmax(c - 1)
                    if c >= 2:
                        emit_store(c - 2)

    nc.compile()
    return nc


def _prep_inputs(x, W1, b1, W2, n_chunks_per_core):
    """Host-side marshalling: bf16 cast, layouts, masks. Returns in_maps."""
    N, H = x.shape
    nodes_per_core = n_chunks_per_core * CHUNK_NODES

    xb = np.asarray(x).astype(NP_BF16)

    # natural layout: [core, chunk, p, t, h]
    x_nat = np.ascontiguousarray(
        xb.reshape(N_CORES, n_chunks_per_core, TILES_PER_CHUNK, TILE_NODES, H)
        .transpose(0, 1, 3, 2, 4))
    # transposed layout: [core, kt, q, n_local]
    x_tr = np.ascontiguousarray(
        xb.reshape(N_CORES, nodes_per_core, H).transpose(0, 2, 1)
        .reshape(N_CORES, 2, 128, nodes_per_core))

    W1b = np.asarray(W1).astype(NP_BF16)     # [hin, hout]
    w1_host = np.ascontiguousarray(
        W1b.reshape(2, 128, 2, 128).transpose(1, 0, 2, 3))  # [p, kt, mt, j]
    w2_host = np.ascontiguousarray(
        np.asarray(W2).astype(NP_BF16).reshape(2, 128).T)   # [p, mt]
    b1_host = np.ascontiguousarray(
        np.asarray(b1).astype(np.float32).reshape(2, 128).T)  # [p, mt]

    # mask[p, g, t] = 1 iff node (t, p) of a chunk belongs to graph g
    p_idx = np.arange(TILE_NODES)
    t_idx = np.arange(TILES_PER_CHUNK)
    g_of_pt = 2 * t_idx[None, :] + p_idx[:, None] // GRAPH_NODES  # [p, t]
    mask_host = (g_of_pt[:, None, :] ==
                 np.arange(CHUNK_GRAPHS)[None, :, None]).astype(NP_BF16)

    in_maps = []
    for core in range(N_CORES):
        in_maps.append({
            "x_nat": x_nat[core],
            "x_tr": x_tr[core],
            "w1": w1_host,
            "w2": w2_host,
            "b1": b1_host,
            "maskw": mask_host,
        })
    return in_maps


def _reference_numpy(x, batch, W1, b1, W2):
    """Fallback for non-uniform batch layouts: straight numpy."""
    x = np.asarray(x, dtype=np.float64)
    batch = np.asarray(batch).astype(np.int64)
    # the reference uses a fixed segment count (num_graphs = num_nodes/64),
    # not batch.max()+1 — keep trailing empty graphs as zero rows
    n_graphs = max(int(batch.max()) + 1, x.shape[0] // GRAPH_NODES)
    scores = np.tanh(x @ np.asarray(W1, np.float64) +
                     np.asarray(b1, np.float64)) @ np.asarray(W2, np.float64)
    scores = scores[:, 0]
    chunk_id = batch // CHUNK_GRAPHS
    n_chunks = int(chunk_id.max()) + 1
    m = np.full(n_chunks, -np.inf)
    np.maximum.at(m, chunk_id, scores)
    e = np.exp(scores - m[chunk_id])
    denom = np.zeros(n_chunks)
    np.add.at(denom, chunk_id, e)
    w = e / denom[chunk_id]
    out = np.zeros((n_graphs, x.shape[1]))
    np.add.at(out, batch, w[:, None] * x)
    return out.astype(np.float32)


def kernel(x, batch, W1, b1, W2, trace=False):
    x = np.asarray(x)
    batch = np.asarray(batch)
    N, H = x.shape
    n_graphs = int(batch[-1]) + 1

    # This kernel is specialized for the uniform sorted batch that the
    # reference generator produces (64 nodes per graph). Anything else
    # falls back to a host computation.
    expected = (np.arange(N, dtype=np.int64) * n_graphs) // N
    if (H != HIDDEN or N % (N_CORES * CHUNK_NODES) != 0
            or n_graphs % (N_CORES * CHUNK_GRAPHS) != 0
            or not np.array_equal(batch.astype(np.int64), expected)):
        return _reference_numpy(x, batch, W1, b1, W2)

    n_chunks_per_core = N // (N_CORES * CHUNK_NODES)

    key = n_chunks_per_core
    if key not in _NC_CACHE:
        _NC_CACHE[key] = build_nc(n_chunks_per_core)
    nc = _NC_CACHE[key]

    in_maps = _prep_inputs(x, W1, b1, W2, n_chunks_per_core)
    try:
        res = run_bass_kernel_spmd(nc, in_maps, core_ids=list(range(N_CORES)),
                                   trace=trace)
    except ModuleNotFoundError:
        # NTFF trace hooks unavailable in this environment
        res = run_bass_kernel_spmd(nc, in_maps, core_ids=list(range(N_CORES)),
                                   trace=False)
    out = np.concatenate([r["out"] for r in res.results], axis=0)
    if trace:
        kernel.last_results = res
    return out.astype(np.float32)
